# revision 1
# baseline (speedup 1.0000x reference)
"""Trainium2 Bass kernel for CustomMultiHeadSelfAttention (fused q/k LayerNorm).

Reference computation (per batch n):
    q = x @ Wq.T ; k = x @ Wk.T ; v = x @ Wv.T          (split into 16 heads of 64)
    q = LN_head(q) * gq + bq ; k = LN_head(k) * gk + bk  (LayerNorm over head_dim)
    out = causal_softmax(q @ k.T) @ v                    (per head)
    y = concat_heads(out) @ Wo.T + bo

Sharding: 8 cores = 2 batches x 4 head-groups (4 heads each).  Each core
computes its heads' attention and a partial y = out_heads @ Wo[:, cols].T;
the host sums the 4 partials per batch and adds bo.

Device-side dataflow per core (all matmuls in fp32r unless noted):
  - QKV projection from streamed xT chunks; LayerNorm mean is folded into
    the Q/K weights on the host (centered weights => mean(q)=0), so LN
    reduces to q * rsqrt(mean(q^2)+eps); *g+b is applied after a PE
    transpose into [d, l] layout (DVE fused mult+add).
  - Scores are computed transposed per head: ST[m,l] = k_ln q_ln^T over
    the causal range l >= 128j, with a -1e30 additive mask on the
    diagonal 128x128 block via an identity matmul, then exp() on ScalarE
    (no max subtraction needed: LN bounds |score| <= 64, exp fits fp32).
  - P = exp(ST) (fp32r).  O^T is accumulated V-stationary:
    OT_aug[65, l] += V_aug[m,65]^T @ P[m,l] where V_aug has a ones column
    appended -> row 64 accumulates the softmax denominators.
  - OT rows are scaled by 1/sums (GPSIMD partition-broadcast) into
    OT_heads, which is directly the lhsT for the output projection.
"""

import numpy as np

import concourse.bass as bass
import concourse.tile as tile
from concourse import bacc, mybir
from concourse.bass_utils import run_bass_kernel_spmd

F32 = mybir.dt.float32
F32R = mybir.dt.float32r
F16 = mybir.dt.float16

P = 128
EMB = 1024
L = 2048
D = 64
HPC = 4           # heads per core
NCORES = 8
EPS = 1e-5
NEG = -1.0e30
T = L // P        # 16 l-tiles
E = EMB // P      # 8 emb chunks
AF = mybir.ActivationFunctionType
ALU = mybir.AluOpType


def _spans(j):
    """Causal spans for m-chunk j: cover [128j, L) with <=1024-wide spans
    whose boundaries (after the first) are 512-aligned."""
    l0 = 128 * j
    c0 = j // 4
    end0 = 512 * (c0 + 1)
    if L - end0 >= 512 and (end0 - l0) + 512 <= 1024:
        end0 += 512
    out = [(l0, end0 - l0)]
    off = end0
    while off < L:
        ln = min(1024, L - off)
        out.append((off, ln))
        off += ln
    return out


def _subs(off, ln):
    """Split [off, off+ln) at 512-multiples."""
    out = []
    cur = off
    while cur < off + ln:
        nxt = min((cur // 512 + 1) * 512, off + ln)
        out.append((cur, nxt - cur))
        cur = nxt
    return out


def build_nc(debug_p=False):
    nc = bacc.Bacc("TRN2", target_bir_lowering=False, debug=False, num_devices=NCORES)

    xT_d = nc.dram_tensor("xT", [EMB, L], F32R, kind="ExternalInput")
    wqv_d = nc.dram_tensor("wqv", [E, P, 512], F32R, kind="ExternalInput")
    wk_d = nc.dram_tensor("wk", [E, P, 256], F32R, kind="ExternalInput")
    wo_d = nc.dram_tensor("wo", [2, P, EMB], F32R, kind="ExternalInput")
    ident_d = nc.dram_tensor("ident", [P, P], F32R, kind="ExternalInput")
    maskf_d = nc.dram_tensor("maskf", [P, P], F32R, kind="ExternalInput")
    gb_d = nc.dram_tensor("gb", [P, 4], F32, kind="ExternalInput")  # gq2 bq2 gk2 bk2
    y_d = nc.dram_tensor("y", [L, EMB], F32, kind="ExternalOutput")
    pdump_d = (nc.dram_tensor("pdump", [T, P, L], F32, kind="ExternalOutput")
               if debug_p else None)

    with tile.TileContext(nc) as tc:
        # ---- persistent pools (bottom of the SBUF stack) ----
        with (
            tc.tile_pool(name="const", bufs=1) as const_p,
            tc.tile_pool(name="vbuf", bufs=1) as vbuf_p,
            tc.tile_pool(name="qtkt", bufs=1) as qtkt_p,
        ):
            ident = const_p.tile([P, P], F32R, tag="ident")
            maskf = const_p.tile([P, P], F32R, tag="maskf")
            gb = const_p.tile([P, 4], F32, tag="gb")
            epst = const_p.tile([P, 1], F32, tag="epst")
            nc.sync.dma_start(ident[:], ident_d[:])
            nc.sync.dma_start(maskf[:], maskf_d[:])
            nc.sync.dma_start(gb[:], gb_d[:])
            nc.vector.memset(epst[:], EPS)

            # V with a ones column per head: vb[t][:, h, 0:65]
            vb = []
            for t in range(T):
                v_ = vbuf_p.tile([P, HPC, 65], F32R, tag=f"vb{t}", name=f"vb{t}")
                nc.gpsimd.memset(v_[:].bitcast(F32), 1.0)  # ones col survives V copy
                vb.append(v_)

            # QT/KT: head pair p occupies rows [0:64]=head 2p, [64:128]=head 2p+1
            qt = [qtkt_p.tile([P, L], F32R, tag=f"qt{p_}", name=f"qt{p_}") for p_ in range(2)]
            kt = [qtkt_p.tile([P, L], F32R, tag=f"kt{p_}", name=f"kt{p_}") for p_ in range(2)]

            # ================= Phase 1: projections + LN + transpose ==========
            with (
                tc.tile_pool(name="xt", bufs=1) as xt_p,
                tc.tile_pool(name="wts", bufs=1) as wts_p,
                tc.tile_pool(name="rows", bufs=4) as rows_p,
                tc.tile_pool(name="stats", bufs=4) as stats_p,
                tc.tile_pool(name="ps_qv", bufs=4, space="PSUM") as ps_qv,
                tc.tile_pool(name="ps_k", bufs=2, space="PSUM") as ps_k,
                tc.tile_pool(name="ps_tr", bufs=2, space="PSUM") as ps_tr,
            ):
                # stream weights + x quarters in the order the proj consumes them
                xtd = xT_d.rearrange("(e p) l -> e p l", p=P)
                wqv, wk = [], []
                xt = [[None] * 4 for _ in range(E)]
                for e in range(E):
                    wq_ = wts_p.tile([P, 512], F32R, tag=f"wqv{e}", name=f"wqv{e}")
                    nc.sync.dma_start(wq_[:], wqv_d[e])
                    wqv.append(wq_)
                    wk_ = wts_p.tile([P, 256], F32R, tag=f"wk{e}", name=f"wk{e}")
                    nc.sync.dma_start(wk_[:], wk_d[e])
                    wk.append(wk_)
                    xe = xt_p.tile([P, 512], F32R, tag=f"xt{e}q0", name=f"xt{e}q0")
                    nc.sync.dma_start(xe[:], xtd[e][:, 0:512])
                    xt[e][0] = xe
                for q in range(1, 4):
                    for e in range(E):
                        xe = xt_p.tile([P, 512], F32R, tag=f"xt{e}q{q}", name=f"xt{e}q{q}")
                        nc.sync.dma_start(xe[:], xtd[e][:, 512 * q:512 * (q + 1)])
                        xt[e][q] = xe

                for t in range(T):
                    pq = ps_qv.tile([P, 512], F32, tag="pqv")
                    pk = ps_k.tile([P, 256], F32, tag="pk")
                    for e in range(E):
                        xchunk = xt[e][t // 4][:, (t % 4) * P:(t % 4 + 1) * P]
                        nc.tensor.matmul(pq[:], xchunk, wqv[e][:],
                                         start=(e == 0), stop=(e == E - 1))
                        nc.tensor.matmul(pk[:], xchunk, wk[e][:],
                                         start=(e == 0), stop=(e == E - 1))
                    # V -> vb[t] (strided into 65-wide head slots), on ACT
                    nc.scalar.copy(
                        vb[t][:, :, 0:64],
                        pq[:, 256:512].rearrange("p (h d) -> p h d", h=HPC))
                    # LN stats: sum of squares per (l, head) for q and k
                    sq = stats_p.tile([P, 512], F32, tag="sq")
                    nc.scalar.activation(sq[:, 0:256], pq[:, 0:256], AF.Square)
                    nc.scalar.activation(sq[:, 256:512], pk[:], AF.Square)
                    ssq = stats_p.tile([P, 8], F32, tag="ssq")
                    nc.vector.tensor_reduce(
                        ssq[:], sq[:].rearrange("p (g d) -> p g d", d=D),
                        axis=mybir.AxisListType.X, op=ALU.add)
                    # rstd = 1/sqrt(ssq/64 + eps); Square/Identity/Sqrt share one
                    # ACT table set (sqrt_and_others) -> one table load for proj
                    std = stats_p.tile([P, 8], F32, tag="std")
                    nc.scalar.activation(std[:], ssq[:], AF.Sqrt,
                                         bias=epst[:], scale=1.0 / D)
                    rstd = stats_p.tile([P, 8], F32, tag="rstd")
                    nc.vector.reciprocal(rstd[:], std[:])
                    # rows = psum * rstd (per head): q on DVE, k on ACT
                    qrow = rows_p.tile([P, 256], F32R, tag="qrow")
                    krow = rows_p.tile([P, 256], F32R, tag="krow")
                    for h in range(HPC):
                        cs = slice(64 * h, 64 * h + 64)
                        nc.vector.tensor_scalar(
                            qrow[:, cs], pq[:, cs], rstd[:, h:h + 1], None, ALU.mult)
                        nc.scalar.activation(
                            krow[:, cs], pk[:, cs], AF.Copy, scale=rstd[:, 4 + h:5 + h])
                    # transpose into [d, l] layout; *g+b fused into PSUM->SBUF copy
                    for p_ in range(2):
                        trq = ps_tr.tile([P, P], F32R, tag="tr")
                        nc.tensor.transpose(trq[:], qrow[:, 128 * p_:128 * (p_ + 1)], ident[:])
                        nc.vector.tensor_scalar(qt[p_][:, t * P:(t + 1) * P], trq[:],
                                                gb[:, 0:1], gb[:, 1:2], ALU.mult, ALU.add)
                        trk = ps_tr.tile([P, P], F32R, tag="tr")
                        nc.tensor.transpose(trk[:], krow[:, 128 * p_:(p_ + 1) * 128], ident[:])
                        nc.vector.tensor_scalar(kt[p_][:, t * P:(t + 1) * P], trk[:],
                                                gb[:, 2:3], gb[:, 3:4], ALU.mult, ALU.add)

            # ================= Phase 2: attention =============================
            with (
                tc.tile_pool(name="wo", bufs=1) as wo_p,
                tc.tile_pool(name="otb", bufs=1) as ot_p,
            ):
                wo = wo_p.tile([P, 2, EMB], F32R, tag="wo")
                nc.sync.dma_start(wo[:], wo_d.rearrange("c p n -> p c n"))
                # OT per (pair, l-chunk): y for quarter c can start as soon as
                # both pairs' chunk c is normalized
                ot = [[ot_p.tile([P, 512], F32R, tag=f"ot{p_}c{c}", name=f"ot{p_}c{c}")
                       for c in range(4)] for p_ in range(2)]

                with (
                    tc.tile_pool(name="pp", bufs=20) as p_pool,
                    tc.tile_pool(name="nrm", bufs=6) as nrm_p,
                    tc.tile_pool(name="ps_s", bufs=2, space="PSUM") as ps_s,
                    tc.tile_pool(name="ps_o", bufs=4, space="PSUM") as ps_o,
                ):
                    # per (pair, head): S -> exp -> immediate O contributions,
                    # j-major so each P span dies right after its O matmuls
                    for p_ in range(2):
                        for hl in range(2):
                            h = 2 * p_ + hl
                            rows = slice(64 * hl, 64 * hl + 64)
                            orows = rows
                            opsum = [ps_o.tile([P, 512], F32, tag="o", name=f"o{c}")
                                     for c in range(4)]
                            def emit_o(j, ptl):
                                # O contributions of j's spans to each l-chunk
                                for (off, ln, pt) in ptl:
                                    for c in range(4):
                                        if (512 * c + 512 <= off
                                                or 512 * c + 512 > off + ln):
                                            continue
                                        start_l = max(512 * c, 128 * j)
                                        if start_l < off:
                                            continue
                                        nc.tensor.matmul(
                                            opsum[c][0:65, start_l - 512 * c:512],
                                            vb[j][:, h, :],
                                            pt[:, start_l - off:512 * c + 512 - off],
                                            start=(j == 0), stop=(j == 4 * c + 3))

                            def emit_norm(c):
                                # normalize rows by the sums row (64), store to OT
                                rec = nrm_p.tile([1, 512], F32, tag="rec")
                                nc.vector.reciprocal(rec[:], opsum[c][64:65, :])
                                recb = nrm_p.tile([64, 512], F32, tag="recb")
                                nc.gpsimd.partition_broadcast(recb[:], rec[:])
                                nc.vector.tensor_tensor(
                                    ot[p_][c][orows, :], opsum[c][0:64, :], recb[:],
                                    ALU.mult)

                            prev = None  # software-pipeline O by one j so the PE
                            for j in range(T):  # never stalls on the exp of j
                                ptl = []
                                for (off, ln) in _spans(j):
                                    # psum columns at 512-aligned positions so no
                                    # matmul output crosses a bank boundary
                                    base = 512 * (off // 512)
                                    sps = ps_s.tile([P, 1024], F32, tag="s")
                                    for (soff, sln) in _subs(off, ln):
                                        # the diagonal block lives in the first sub
                                        # of the span that starts at 128j
                                        diag = soff == off == 128 * j
                                        nc.tensor.matmul(
                                            sps[:, soff - base:soff - base + sln],
                                            kt[p_][rows, j * P:(j + 1) * P],
                                            qt[p_][rows, soff:soff + sln],
                                            start=True, stop=not diag)
                                        if diag:
                                            # diag 128x128 causal mask (-1e30)
                                            nc.tensor.matmul(
                                                sps[:, off - base:off - base + P],
                                                ident[:], maskf[:],
                                                start=False, stop=True)
                                    pt = p_pool.tile([P, 1024], F32R, tag="p")
                                    nc.scalar.activation(
                                        pt[:, :ln], sps[:, off - base:off - base + ln],
                                        AF.Exp)
                                    if pdump_d is not None and p_ == 0 and hl == 0:
                                        nc.sync.dma_start(
                                            pdump_d[j, :, off:off + ln],
                                            pt[:, :ln].bitcast(F32))
                                    ptl.append((off, ln, pt))
                                if prev is not None:
                                    emit_o(*prev)
                                    # chunk c is complete once j=4c+3's O landed:
                                    # normalize immediately to free its psum bank
                                    if prev[0] % 4 == 3:
                                        emit_norm(prev[0] // 4)
                                prev = (j, ptl)
                            emit_o(*prev)
                            emit_norm(3)

                # ---- output projection: y[t] = sum_p OT_p[:,t]^T @ WoT_p ----
                with (
                    tc.tile_pool(name="ysb", bufs=12) as ysb_p,
                    tc.tile_pool(name="ps_y", bufs=8, space="PSUM") as ps_y,
                ):
                    for t in range(T):
                        for eh in range(2):
                            yps = ps_y.tile([P, 512], F32, tag="y")
                            for p_ in range(2):
                                nc.tensor.matmul(
                                    yps[:], ot[p_][t // 4][:, (t % 4) * P:(t % 4 + 1) * P],
                                    wo[:, p_, 512 * eh:512 * (eh + 1)],
                                    start=(p_ == 0), stop=(p_ == 1))
                            ysb = ysb_p.tile([P, 512], F32, tag="ysb")
                            nc.scalar.copy(ysb[:], yps[:])
                            nc.sync.dma_start(
                                y_d[t * P:(t + 1) * P, 512 * eh:512 * (eh + 1)], ysb[:])

    nc.compile()
    return nc


_NC = None


def _get_nc():
    global _NC
    if _NC is None:
        _NC = build_nc()
    return _NC


def _center(w):
    # fold LayerNorm mean-subtraction into the projection weights (per head)
    w3 = w.astype(np.float64).reshape(-1, D, EMB)
    w3 = w3 - w3.mean(axis=1, keepdims=True)
    return w3.reshape(-1, EMB).astype(np.float32)


def make_in_maps(x, Wq, Wk, Wv, gq, bq, gk, bk, Wo):
    x = np.asarray(x, np.float32)
    Wq = np.asarray(Wq, np.float32)
    Wk = np.asarray(Wk, np.float32)
    Wv = np.asarray(Wv, np.float32)
    Wo = np.asarray(Wo, np.float32)
    gq = np.asarray(gq, np.float32)
    bq = np.asarray(bq, np.float32)
    gk = np.asarray(gk, np.float32)
    bk = np.asarray(bk, np.float32)

    ident = np.eye(P, dtype=np.float32)
    # additive causal mask for the diagonal 128x128 block of ST[m, l_local]:
    # invalid where l < m
    maskf = np.where(np.arange(P)[None, :] < np.arange(P)[:, None], NEG, 0.0
                     ).astype(np.float32)
    gb = np.stack([np.tile(gq, 2), np.tile(bq, 2), np.tile(gk, 2), np.tile(bk, 2)],
                  axis=1).astype(np.float32)  # [128, 4]

    in_maps = []
    for c in range(NCORES):
        n, g = divmod(c, HPC)
        rows = slice(256 * g, 256 * (g + 1))
        xT = np.ascontiguousarray(x[n].T)
        wqT = np.ascontiguousarray(_center(Wq[rows]).T).reshape(E, P, 256)
        wvT = np.ascontiguousarray(Wv[rows].T).reshape(E, P, 256)
        wqv = np.concatenate([wqT, wvT], axis=2)
        wkT = np.ascontiguousarray(_center(Wk[rows]).T).reshape(E, P, 256)
        woT = np.ascontiguousarray(Wo[:, rows].T).reshape(2, P, EMB)
        in_maps.append({
            "xT": xT, "wqv": np.ascontiguousarray(wqv), "wk": wkT, "wo": woT,
            "ident": ident, "maskf": maskf, "gb": gb,
        })
    return in_maps


def kernel(x, mask, Wq, Wk, Wv, gq, bq, gk, bk, Wo, bo):
    nc = _get_nc()
    in_maps = make_in_maps(x, Wq, Wk, Wv, gq, bq, gk, bk, Wo)
    res = run_bass_kernel_spmd(nc, in_maps, list(range(NCORES)))
    bo = np.asarray(bo, np.float32)
    y = np.zeros((2, L, EMB), np.float32)
    for n in range(2):
        acc = np.zeros((L, EMB), np.float32)
        for g in range(HPC):
            acc += res.results[HPC * n + g]["y"]
        y[n] = acc + bo[None, :]
    return y

